# revision 41
# baseline (speedup 1.0000x reference)
"""Causal self-attention on 8 trn2 NeuronCores.

Sharding: tensor-parallel over heads. Core c computes Q/K/V and attention
for heads {2c, 2c+1} over all batches (column-parallel W_q/W_k/W_v slices),
then a per-batch 8-rank AllToAll redistributes the per-head attention
outputs so each core runs the full output projection (row-parallel
contraction over all 16 heads' features) for its 1024-token chunk.

Implementation notes (per core):
 - Q/K projections run in fp8e4 with DoubleRow perf mode (weights host
   prescaled by WS and pre-interleaved).
 - V projection: in-batch key blocks 0-1 (tokens < 256, the only ones an
   early low-averaging query can see) use a 3-pass fp8 error-compensated
   product into bf16 V_sb; blocks >= 2 use a single fp8 pass into fp8 V8
   (their quantization error is averaged down by >=257-key softmax sums).
 - Scores are computed transposed (scoresT [key part, q free]) into a
   two-head PSUM tile; the causal upper triangle is handled by an
   identity-matmul accumulating a -1e9 block; fully masked column ranges
   are never computed or read.
 - Softmax skips max-subtraction (scores are O(1)); probabilities carry an
   exp bias of EXPB that cancels in the normalization.
 - Scores also run as fp8 DoubleRow matmuls: Q and K are stored fp8 in an
   interleaved [64 part, 2 slot, tok] layout (feature 64h + 32s + p on
   partition 32h + p) so each head's 64-deep contraction packs into 32
   partitions x 2 slots at 0.5 cycles/row.
 - Probabilities for q-tile 0 blocks 0-1 are bf16 (P2b); everything else
   is fp8e4 (P28). One exp instruction per key-block covers both heads,
   alternating between the Scalar engine (exact LUT exp, fp8/bf16 out)
   and the Vector engine (Schraudolph bit-pattern exp: int16 affine for
   bf16, uint8 affine for fp8e4; uint8 saturation maps masked scores to
   +0 and the fp8e4-inf cliff sits at score 6.3, far above the observed
   max ~2.9).
 - P@V is computed transposed per 128-query chunk: out [128 q, 72] with
   column 64 = softmax denominator (ones column in V, pad cols zero so
   the DoubleRow pair stride stays 16B-aligned). q-tile 0 uses bf16
   matmuls for blocks 0-1; all other blocks go two-at-a-time as fp8
   DoubleRow matmuls (contraction 256 at 0.5 cycles/row), odd tail as a
   plain fp8 matmul. One reciprocal + one broadcast-multiply normalize
   two chunks at once.
 - Each unit's Q/K/V projection + staging copies are emitted after the
   PREVIOUS unit's P@V (software pipelining): the projection matmuls run
   on the PE during the previous unit's softmax-exp drain, and the copies
   land on ACT/DVE right behind its exps instead of behind its
   normalization.
 - Attention chunks are transposed back to feature-major on the PE,
   staged to DRAM, AllToAll'd per batch, and projected with W_o; the
   output projection matmuls are interleaved into the next batch's
   attention as PE filler work, with all of batch B-2's units reserved
   to fill the final AllToAll window.
 - b_o (and the W_o @ b_v correction) are added on the host after the
   gather; the 1/WS dequant scale is folded into the W_o weights.
"""

import numpy as np
import ml_dtypes

import concourse.bass as bass
import concourse.mybir as mybir
import concourse.tile as tile
from concourse import bacc
from concourse.bass_utils import run_bass_kernel_spmd

B, L, D, H, HD = 4, 2048, 1024, 16, 64
NCORES = 8
DL = 128              # local feature dim: 2 heads * 64
BL = B * L            # 8192
CHUNK = BL // NCORES  # 1024 output rows per core
QB = 256              # tokens per core per A2A quarter (one per batch)
SCALE = HD ** -0.5
NEG = -1e9
WS = 32.0             # weight prescale (fp8 range)
EXPB = -0.75          # exp bias: P *= e^EXPB, cancels in normalization
DQ = SCALE / (WS * WS)

# Schraudolph fast-exp constants (bf16 bit pattern via int16 affine)
A16 = 128.0 / np.log(2.0)
C16 = 6.0
B16 = 127.0 * 128.0 - C16 + A16 * EXPB
# fp8e4 (bias-7) bit pattern via uint8 affine
A8 = 8.0 / np.log(2.0)
C8 = 0.45
B8 = 7.0 * 8.0 - C8 + A8 * EXPB

# exp engine split pattern (D = DVE schraudolph, A = ACT exact exp)
EXP_PAT = "ADADADADADADADAD"

QT = 512              # query tile
KB = 128              # key block
NQT = L // QT         # 4 q-tiles per batch
NKB = L // KB         # 16 k-blocks per batch
ND = D // 128         # 8 d_model partition tiles
VW = 72               # padded P@V width: 64 V cols + 1 denom + 7 pad

FP32 = mybir.dt.float32
BF16 = mybir.dt.bfloat16
F8E4 = mybir.dt.float8e4
I16 = mybir.dt.int16
U8 = mybir.dt.uint8
EXP = mybir.ActivationFunctionType.Exp
IDENT = mybir.ActivationFunctionType.Identity
COPY = mybir.ActivationFunctionType.Copy
MULT = mybir.AluOpType.mult
ADD = mybir.AluOpType.add
DR = mybir.MatmulPerfMode.DoubleRow

TRACE = False
LAST_EXEC_NS = None
_CACHED_NC = None
_SIM_MODE = False   # replace the collective with a local DMA; 1 device


def build_program():
    nc = bacc.Bacc("TRN2", target_bir_lowering=False, debug=False,
                   num_devices=(1 if _SIM_MODE else NCORES))
    x8T = nc.dram_tensor("x8T", [D, BL], F8E4, kind="ExternalInput").ap()
    wq8 = nc.dram_tensor("wq8", [D, DL], F8E4, kind="ExternalInput").ap()
    wk8 = nc.dram_tensor("wk8", [D, DL], F8E4, kind="ExternalInput").ap()
    xlr = nc.dram_tensor("x8l", [D, B * QB], F8E4, kind="ExternalInput").ap()
    wv8h = nc.dram_tensor("wv8h", [D, DL], F8E4, kind="ExternalInput").ap()
    wv8l = nc.dram_tensor("wv8l", [D, DL], F8E4, kind="ExternalInput").ap()
    wob = nc.dram_tensor("wob", [D, D], BF16, kind="ExternalInput").ap()
    bq_c = nc.dram_tensor("bq_c", [64, 2], FP32, kind="ExternalInput").ap()
    bk_c = nc.dram_tensor("bk_c", [64, 2], FP32, kind="ExternalInput").ap()
    pad_a = nc.dram_tensor("pad_a", [KB, B * NKB], FP32, kind="ExternalInput").ap()
    pad_s = nc.dram_tensor("pad_s", [KB, B * NKB], FP32, kind="ExternalInput").ap()
    pad_u = nc.dram_tensor("pad_u", [KB, B * NKB], FP32, kind="ExternalInput").ap()
    tri_d = nc.dram_tensor("tri_d", [KB, KB], BF16, kind="ExternalInput").ap()
    id_d = nc.dram_tensor("id_d", [KB, KB], BF16, kind="ExternalInput").ap()
    outT = nc.dram_tensor("outT", [D, CHUNK], FP32, kind="ExternalOutput").ap()

    with tile.TileContext(nc) as tc:
        with tc.tile_pool(name="persist", bufs=1) as persist, \
             tc.tile_pool(name="x8p", bufs=4) as x8p, \
             tc.tile_pool(name="xlp", bufs=2) as xlp, \
             tc.tile_pool(name="p2b", bufs=2) as p2bp, \
             tc.tile_pool(name="p28", bufs=2) as p28p, \
             tc.tile_pool(name="attp", bufs=3) as attp, \
             tc.tile_pool(name="afm", bufs=3) as afm, \
             tc.tile_pool(name="gathp", bufs=2) as gathp, \
             tc.tile_pool(name="otp", bufs=3) as otp, \
             tc.tile_pool(name="small", bufs=4) as small, \
             tc.tile_pool(name="ps01", bufs=2, space="PSUM") as ps01p, \
             tc.tile_pool(name="proj", bufs=1, space="PSUM") as projp, \
             tc.tile_pool(name="mix", bufs=2, space="PSUM") as mixp, \
             tc.tile_pool(name="dram", bufs=1, space="DRAM") as dram:

            # ---- weights/constants; emission order = SP DMA issue order ----
            wq_sb = persist.tile([128, ND // 2, 2, 2, 64], F8E4)
            wk_sb = persist.tile([128, ND // 2, 2, 2, 64], F8E4)
            nc.sync.dma_start(out=wq_sb, in_=wq8.rearrange(
                "(dp j p) (s m) -> p dp j s m", p=128, j=2, s=2))
            nc.sync.dma_start(out=wk_sb, in_=wk8.rearrange(
                "(dp j p) (s m) -> p dp j s m", p=128, j=2, s=2))
            wvh_sb = persist.tile([128, ND // 2, 2, DL], F8E4)
            wvl_sb = persist.tile([128, ND // 2, 2, DL], F8E4)
            wo_sb = persist.tile([128, ND, D], BF16)   # loaded during b0 attn
            bq_sb = persist.tile([64, 2], FP32)
            bk_sb = persist.tile([64, 2], FP32)
            pada_sb = persist.tile([KB, B * NKB], FP32)
            pads_sb = persist.tile([KB, B * NKB], FP32)
            padu_sb = persist.tile([KB, B * NKB], FP32)
            tri_sb = persist.tile([KB, KB], BF16)
            id_sb = persist.tile([KB, KB], BF16)

            # ---- persistent activations ----
            # interleaved fp8 Q/K: partition 32h+p, slot s = feature
            # 64h + 32s + p, values *WS (for DoubleRow score matmuls)
            QI_sb = persist.tile([64, 2, BL], F8E4)
            KI_sb = persist.tile([64, 2, BL], F8E4)
            # V for in-batch key blocks 0-1: bf16, error-compensated
            Vb_sb = persist.tile([128, B, 2, 2, VW], BF16)  # [tok, b, blk, h, V|1|0pad]
            # V for in-batch key blocks 2-15: fp8 single-pass
            V8_sb = persist.tile([128, B * NKB, 2, VW], F8E4)
            # data cols [0:64) are fully written before any read; only the
            # denominator (64) and pad (65:) columns need initialization
            nc.gpsimd.memset(Vb_sb[:, :, :, :, 64:VW], 0.0)
            nc.gpsimd.memset(V8_sb[:, :, :, 64:VW], 0.0)
            nc.gpsimd.memset(Vb_sb[:, :, :, :, 64:65], 1.0)  # denominator col
            nc.gpsimd.memset(V8_sb[:, :, :, 64:65], 1.0)

            # PE p-state warmup on zeroed data while the first DMAs land,
            # and the exp activation table load off the critical path
            warm_sb = persist.tile([128, 512], BF16)
            nc.vector.memset(warm_sb, 0.0)
            warm_ps = mixp.tile([128, 512], FP32, tag="mix")
            for w in range(8):
                nc.tensor.matmul(warm_ps, lhsT=warm_sb[:, 0:128], rhs=warm_sb,
                                 start=(w == 0), stop=(w == 7))
            nc.scalar.activation(warm_sb[0:1, 0:1], warm_ps[0:1, 0:1], EXP)

            a2a_in = [None] * B
            a2a_out = [None] * B
            for p in range(B):
                a2a_in[p] = dram.tile([NCORES * 128, QB], BF16,
                                      tag=f"a2a_in{p}", name=f"a2a_in{p}")
                a2a_out[p] = dram.tile([NCORES * 128, QB], BF16,
                                       tag=f"a2a_out{p}", name=f"a2a_out{p}")

            ebusy = {"A": 0.0, "D": 0.0}   # estimated ACT/DVE busy ns

            def exp_engine(nelem):
                ca = nelem * 0.8333 + 185.0
                cd = nelem * 1.0417 + 125.0
                if ebusy["A"] + ca * 0.5 <= ebusy["D"] + cd * 0.5:
                    ebusy["A"] += ca
                    return "A"
                ebusy["D"] += cd
                return "D"

            pending = [None]   # (attT, b, qt) awaiting transpose+staging
            gath_t = [None]    # gather tile of the in-flight out-projection

            def flush_stage(dma_q=None):
                if pending[0] is None:
                    return
                attT_p, b_p, qt_p = pending[0]
                pending[0] = None
                tp = mixp.tile([128, 2, 2, KB], BF16, tag="mix")
                for qc in range(4):
                    nc.tensor.transpose(tp[:, qc // 2, qc % 2, :],
                                        attT_p[:, qc], id_sb)
                af = afm.tile([128, 2, 2, KB], BF16, tag="af")
                nc.vector.tensor_copy(af, tp)
                ebusy["D"] += 658.0
                (dma_q or nc.sync).dma_start(
                    out=a2a_in[b_p][256 * qt_p:256 * (qt_p + 1), :].rearrange(
                        "(jj p) t -> p jj t", p=128),
                    in_=af)

            def a2a_start(p):
                """Launch A2A for batch p and the gather of its result."""
                gath = gathp.tile([128, NCORES, QB], BF16, tag="gath")
                dq = nc.sync
                if _SIM_MODE:
                    # collective bypass on one core is the identity; gather
                    # straight from the staging buffer
                    dq.dma_start(out=gath, in_=a2a_in[p].rearrange(
                        "(j p) t -> p j t", p=128))
                else:
                    nc.gpsimd.collective_compute(
                        "AllToAll", mybir.AluOpType.bypass,
                        replica_groups=[list(range(NCORES))],
                        ins=[a2a_in[p].opt()], outs=[a2a_out[p].opt()])
                    dq.dma_start(out=gath, in_=a2a_out[p].rearrange(
                        "(j p) t -> p j t", p=128))
                gath_t[0] = gath

            def out_proj_unit(p, dt, eng="act"):
                """One [128 dfeat, 256 tok] slice of batch p's projection."""
                gath = gath_t[0]
                ps_o = mixp.tile([128, QB], FP32, tag="mix")
                for vt in range(ND):
                    nc.tensor.matmul(ps_o,
                                     lhsT=wo_sb[:, vt, KB * dt:KB * (dt + 1)],
                                     rhs=gath[:, vt, :],
                                     start=(vt == 0), stop=(vt == ND - 1))
                ot = otp.tile([128, QB], FP32, tag="ot")
                if eng == "act":
                    nc.scalar.activation(ot, ps_o, COPY)
                    ebusy["A"] += 398.0
                else:
                    nc.vector.tensor_copy(ot, ps_o)
                    ebusy["D"] += 392.0
                nc.sync.dma_start(
                    out=outT[KB * dt:KB * (dt + 1), QB * p:QB * (p + 1)],
                    in_=ot)

            for b in range(B):
                # ---- interleaved QKV (lc=u) + attention (qt=u) units ----
                for u in range(NQT):
                    lc = u
                    t0 = L * b + QT * lc
                    if b > 0 and lc == 0:
                        flush_stage()          # staging for (b-1, qt=3)
                        a2a_start(b - 1)
                    x8 = x8p.tile([128, ND // 2, 2, QT], F8E4, tag="x8")
                    nc.sync.dma_start(out=x8, in_=x8T[:, t0:t0 + QT].rearrange(
                        "(dp j p) l -> p dp j l", p=128, j=2))
                    if lc == 0:
                        # xl: fp8 residual of x, only tokens [0,256) of batch b
                        xl = xlp.tile([128, ND // 2, 2, QB], F8E4, tag="xl")
                        nc.sync.dma_start(
                            out=xl, in_=xlr[:, QB * b:QB * (b + 1)].rearrange(
                                "(dp j p) l -> p dp j l", p=128, j=2))
                    if b == 0 and lc == 0:
                        # deferred small constants: after the first x tiles
                        nc.sync.dma_start(out=wvh_sb, in_=wv8h.rearrange(
                            "(dp j p) m -> p dp j m", p=128, j=2))
                        nc.sync.dma_start(out=wvl_sb, in_=wv8l.rearrange(
                            "(dp j p) m -> p dp j m", p=128, j=2))
                        nc.sync.dma_start(out=bq_sb, in_=bq_c)
                        nc.sync.dma_start(out=bk_sb, in_=bk_c)
                        nc.sync.dma_start(out=pada_sb, in_=pad_a)
                        nc.sync.dma_start(out=pads_sb, in_=pad_s)
                        nc.sync.dma_start(out=padu_sb, in_=pad_u)
                        nc.sync.dma_start(out=tri_sb, in_=tri_d)
                        nc.sync.dma_start(out=id_sb, in_=id_d)
                    pst = projp.tile([128, 2, QT], FP32, tag="proj")
                    ps_q = pst[0:64]
                    ps_k = pst[64:128]
                    for s in range(2):
                        for dp in range(ND // 2):
                            nc.tensor.matmul(ps_q[:, s, :],
                                             lhsT=wq_sb[:, dp, :, s, :],
                                             rhs=x8[:, dp], perf_mode=DR,
                                             start=(dp == 0),
                                             stop=(dp == ND // 2 - 1))
                    for s in range(2):
                        for dp in range(ND // 2):
                            nc.tensor.matmul(ps_k[:, s, :],
                                             lhsT=wk_sb[:, dp, :, s, :],
                                             rhs=x8[:, dp], perf_mode=DR,
                                             start=(dp == 0),
                                             stop=(dp == ND // 2 - 1))
                    for s in range(2):
                        nc.scalar.activation(QI_sb[:, s, t0:t0 + QT],
                                             ps_q[:, s, :], IDENT,
                                             bias=bq_sb[:, s:s + 1], scale=1.0)
                        nc.vector.tensor_scalar_add(KI_sb[:, s, t0:t0 + QT],
                                                    ps_k[:, s, :],
                                                    bk_sb[:, s:s + 1])
                    ebusy["A"] += 2 * 612.0
                    ebusy["D"] += 2 * 658.0 + 530.0   # KI copies + V copy
                    ps_v = mixp.tile([128, 4, 2, 64], FP32, tag="mix")
                    for vs in range(QT // KB):
                        vsl = slice(KB * vs, KB * (vs + 1))
                        three_pass = (lc == 0 and vs < 2)
                        for dp in range(ND // 2):
                            nc.tensor.matmul(ps_v[:, vs], perf_mode=DR,
                                             lhsT=x8[:, dp, :, vsl],
                                             rhs=wvh_sb[:, dp],
                                             start=(dp == 0),
                                             stop=(not three_pass
                                                   and dp == ND // 2 - 1))
                        if three_pass:
                            for dp in range(ND // 2):
                                nc.tensor.matmul(
                                    ps_v[:, vs], perf_mode=DR,
                                    lhsT=xl[:, dp, :, vsl],
                                    rhs=wvh_sb[:, dp],
                                    start=False, stop=False)
                            for dp in range(ND // 2):
                                nc.tensor.matmul(
                                    ps_v[:, vs], perf_mode=DR,
                                    lhsT=x8[:, dp, :, vsl],
                                    rhs=wvl_sb[:, dp],
                                    start=False, stop=(dp == ND // 2 - 1))
                    kt0v = NKB * b + 4 * lc
                    if lc == 0:
                        nc.vector.tensor_copy(Vb_sb[:, b, :, :, 0:64],
                                              ps_v[:, 0:2])
                        nc.vector.tensor_copy(V8_sb[:, kt0v:kt0v + 4, :, 0:64],
                                              ps_v)
                    else:
                        nc.vector.tensor_copy(V8_sb[:, kt0v:kt0v + 4, :, 0:64],
                                              ps_v)

                    # -- attention q-tile qt=u (+ out-proj filler for b-1) --
                    qt = u
                    nkb = 4 * (qt + 1)
                    q0 = L * b + QT * qt
                    P2b = p2bp.tile([128, 2, 2, QT], BF16, tag="p2b")
                    P28 = p28p.tile([128, NKB, 2, QT], F8E4, tag="p28")
                    P2bi = P2b.bitcast(I16)
                    P28u = P28.bitcast(U8)
                    attT = attp.tile([128, NQT, 2, 64], BF16, tag="attT")

                    def pv_pair(qp):
                        # P@V for query chunks 2qp, 2qp+1; one reciprocal and
                        # one broadcast-multiply normalize both chunks
                        pvT = mixp.tile([128, 2, 2, VW], FP32, tag="mix")
                        for qi in range(2):
                            qc = 2 * qp + qi
                            jmax = 4 * qt + qc
                            qsl = slice(KB * qc, KB * (qc + 1))
                            jf0 = 2 if qt == 0 else 0
                            nb = min(jmax + 1, jf0)   # bf16 blocks (qt=0 only)
                            nf = jmax + 1 - nb        # fp8 blocks from j=jf0
                            for h in range(2):
                                first = True
                                for jj in range(nb):
                                    nc.tensor.matmul(
                                        pvT[:, qi, h, :],
                                        lhsT=P2b[:, jj, h, qsl],
                                        rhs=Vb_sb[:, b, jj, h, :],
                                        start=first,
                                        stop=(nf == 0 and jj == jmax))
                                    first = False
                                for m in range(nf // 2):
                                    j = jf0 + 2 * m
                                    nc.tensor.matmul(
                                        pvT[:, qi, h, :],
                                        lhsT=P28[:, j:j + 2, h, qsl],
                                        rhs=V8_sb[:, NKB * b + j:NKB * b + j + 2, h, :],
                                        perf_mode=DR,
                                        start=first,
                                        stop=(nf % 2 == 0 and j + 1 == jmax))
                                    first = False
                                if nf % 2:
                                    nc.tensor.matmul(
                                        pvT[:, qi, h, :],
                                        lhsT=P28[:, jmax, h, qsl],
                                        rhs=V8_sb[:, NKB * b + jmax, h, :],
                                        start=first, stop=True)
                        rec = small.tile([128, 2, 2, 1], FP32, tag="rec")
                        nc.vector.reciprocal(rec, pvT[:, :, :, 64:65])
                        nc.vector.tensor_tensor(
                            attT[:, 2 * qp:2 * qp + 2, :, :],
                            pvT[:, :, :, 0:64],
                            rec.broadcast_to([128, 2, 2, 64]), MULT)
                        ebusy["D"] += 129.0 + 392.0

                    for j in range(nkb):
                        kt = NKB * b + j
                        k0 = L * b + KB * j
                        o = j - 4 * qt
                        c0 = max(0, KB * o)
                        psh = ps01p.tile([128, 2, QT], FP32, tag="ps01")
                        for h in range(2):
                            nc.tensor.matmul(
                                psh[:, h, c0:QT],
                                lhsT=KI_sb[32 * h:32 * (h + 1), :, k0:k0 + KB],
                                rhs=QI_sb[32 * h:32 * (h + 1), :,
                                          q0 + c0:q0 + QT],
                                perf_mode=DR, start=True, stop=(o < 0))
                            if o >= 0:
                                nc.tensor.matmul(psh[:, h, c0:c0 + KB],
                                                 lhsT=id_sb, rhs=tri_sb,
                                                 start=False, stop=True)
                        if o >= 0:
                            # diagonal blocks by size: big ones to the faster
                            # ACT, small ones to DVE, balancing each unit
                            eng = "A" if o < 2 else "D"
                        else:
                            eng = exp_engine(2 * (QT - c0))
                        if qt == 0 and j < 2:
                            if eng == "D":
                                nc.vector.tensor_scalar(
                                    P2bi[:, j, :, c0:QT], psh[:, :, c0:QT],
                                    A16 * DQ, pads_sb[:, kt:kt + 1], MULT, ADD)
                            else:
                                nc.scalar.activation(
                                    P2b[:, j, :, c0:QT], psh[:, :, c0:QT], EXP,
                                    bias=pada_sb[:, kt:kt + 1], scale=DQ)
                        else:
                            if eng == "D":
                                nc.vector.tensor_scalar(
                                    P28u[:, j, :, c0:QT], psh[:, :, c0:QT],
                                    A8 * DQ, padu_sb[:, kt:kt + 1], MULT, ADD)
                            else:
                                nc.scalar.activation(
                                    P28[:, j, :, c0:QT], psh[:, :, c0:QT], EXP,
                                    bias=pada_sb[:, kt:kt + 1], scale=DQ)
                    if 2 <= g <= 5:            # W_o load in 4 chunks, after
                        wc = g - 2                 # the start-up DMA burst
                        nc.sync.dma_start(
                            out=wo_sb[:, 2 * wc:2 * (wc + 1), :],
                            in_=wob[QB * wc:QB * (wc + 1), :].rearrange(
                                "(t p) m -> p t m", p=128))
                    if b > 0 and qt >= 2 and b < B - 1:  # PE filler units
                        for dt in range(4 * (qt - 2), 4 * (qt - 1)):
                            out_proj_unit(b - 1, dt)
                    elif b == B - 1 and qt >= 2:
                        for dt in range(2 * (qt - 2), 2 * (qt - 1)):
                            out_proj_unit(b - 1, dt)
                    for qp in range(2):
                        pv_pair(qp)
                    flush_stage()              # staging for (b, qt-1)
                    pending[0] = (attT, b, qt)
            flush_stage(dma_q=nc.sync)
            for dt in range(4, ND):    # remaining b2 units fill the a2a gap
                out_proj_unit(B - 2, dt, eng=("act" if dt % 2 else "dve"))
            a2a_start(B - 1)
            for dt in range(ND):
                out_proj_unit(B - 1, dt, eng=("act" if dt % 2 else "dve"))

    nc.compile()
    return nc


def kernel(x, mask, W_q, b_q, W_k, b_k, W_v, b_v, W_o, b_o):
    global _CACHED_NC, LAST_EXEC_NS
    bf16 = ml_dtypes.bfloat16
    f8 = ml_dtypes.float8_e4m3
    x = np.asarray(x, np.float32)
    mask = np.asarray(mask)

    xT = np.ascontiguousarray(x.reshape(BL, D).T)
    x8T = xT.astype(f8)
    # fp8 residual of x, only the first 256 tokens of each batch (the only
    # tokens whose V needs the 3-pass compensated product)
    xl_cols = np.concatenate(
        [np.arange(L * b, L * b + QB) for b in range(B)])
    x8l_h = np.ascontiguousarray(
        (xT[:, xl_cols] - x8T[:, xl_cols].astype(np.float32))).astype(f8)
    wob = np.ascontiguousarray(
        np.asarray(W_o, np.float32).T / WS).astype(bf16)
    bo_full = (np.asarray(b_o, np.float32)
               + np.asarray(W_o, np.float32) @ np.asarray(b_v, np.float32))
    pb = np.where(mask != 0, 0.0, NEG).astype(np.float32)        # [B, L]
    pad = np.ascontiguousarray(
        pb.reshape(B, NKB, KB).transpose(2, 0, 1).reshape(KB, B * NKB))
    pad_a = (EXPB + pad).astype(np.float32)
    pad_s = (B16 + A16 * pad).astype(np.float32)
    pad_u = (B8 + A8 * pad).astype(np.float32)
    wv8h_c = []
    wv8l_c = []
    for c in range(NCORES):
        sl = slice(DL * c, DL * (c + 1))
        wvs = np.ascontiguousarray(np.asarray(W_v, np.float32)[sl].T * WS)
        wh = wvs.astype(f8)
        wv8h_c.append(wh)
        wv8l_c.append((wvs - wh.astype(np.float32)).astype(f8))
    # column permutation for the interleaved Q/K layout: host col s*64+p
    # holds local feature 64*(p//32) + 32*s + (p%32)
    perm = np.array([64 * (p // 32) + 32 * s + (p % 32)
                     for s in (0, 1) for p in range(64)])
    kp = np.arange(KB)[:, None]
    qs = np.arange(KB)[None, :]
    tri = np.where(kp > qs, NEG, 0.0).astype(bf16)
    id128 = np.eye(KB, dtype=np.float32).astype(bf16)

    in_maps = []
    for c in range(NCORES):
        sl = slice(DL * c, DL * (c + 1))
        in_maps.append({
            "x8T": x8T, "x8l": x8l_h, "wob": wob,
            "pad_a": pad_a, "pad_s": pad_s, "pad_u": pad_u,
            "tri_d": tri, "id_d": id128,
            "wq8": np.ascontiguousarray(
                (np.asarray(W_q, np.float32)[sl].T * WS)[:, perm]).astype(f8),
            "wk8": np.ascontiguousarray(
                (np.asarray(W_k, np.float32)[sl].T * WS)[:, perm]).astype(f8),
            "wv8h": wv8h_c[c], "wv8l": wv8l_c[c],
            "bq_c": np.ascontiguousarray(
                (np.asarray(b_q, np.float32)[sl] * WS)[perm]
                .reshape(2, 64).T),
            "bk_c": np.ascontiguousarray(
                (np.asarray(b_k, np.float32)[sl] * WS)[perm]
                .reshape(2, 64).T),
        })

    if _CACHED_NC is None:
        _CACHED_NC = build_program()
    res = run_bass_kernel_spmd(_CACHED_NC, in_maps, list(range(NCORES)),
                               trace=TRACE)
    LAST_EXEC_NS = res.exec_time_ns
    # core c's outT [D, CHUNK]: quarter b columns are tokens 2048b + 256c + i
    out = np.empty((BL, D), np.float32)
    for c in range(NCORES):
        oc = res.results[c]["outT"]
        for b in range(B):
            out[L * b + QB * c:L * b + QB * (c + 1)] = \
                oc[:, QB * b:QB * (b + 1)].T
    out += bo_full[None, :]
    return np.ascontiguousarray(out.reshape(B, L, D))
